# revision 30
# baseline (speedup 1.0000x reference)
"""Trainium2 Bass kernel for Mistral sliding-window attention (B=1, S=4096,
HID=1024, H=8 q-heads, KVH=2 kv-heads, D=128, WINDOW=2048).

Strategy: shard the 4096-token sequence across 8 NeuronCores (512 queries per
core). Each core recomputes K/V projections for its 2560-row key window
(own 512 rows + previous 2048), applies RoPE, computes sliding-window causal
attention for all 8 heads in the [keys, queries] orientation (scores^T), and
runs the full o_proj for its 512 rows. No collectives; the host concatenates
the per-core 512-row output blocks.

Precision plan (validated in numpy + HW smoke tests):
- "recent" keys (the core's own 512 rows, chunks 16..19) stay in fp16
  end-to-end: this region dominates accuracy for short-window (early)
  queries where softmax is concentrated.
- "old" keys (previous 2048 rows, chunks 0..15) run in fp8-e4m3 with
  DoubleRow matmuls (2 rows/cycle):
    * K/V projections from fp8 x / fp8 weights, pairing hidden chunks,
    * scores with d packed [64 partitions x 2 pairs],
    * exp P written directly as fp8 by the activation engine,
    * PV and the softmax-denominator ones-matmul pair CHUNKS via the
      [128, 2, 512] P layout -> one pass per two chunks (4x cheaper).
- Q and O projections fp16.

Softmax without max-subtraction (scores are O(1)); the denominator comes
from ones-matmuls, corrected for zero-padded keys via a host-provided
count (padded keys contribute exp(0)=1 exactly, even in fp8).
"""

import sys
import numpy as np
from contextlib import ExitStack

if "/opt/trn_rl_repo" not in sys.path:
    sys.path.insert(0, "/opt/trn_rl_repo")

import ml_dtypes

# tuning knobs resolved at build time (A/B'd via TimelineSim)
TUNE = dict(order_old_first=False, one_ahead=True, chunked_x16=True, o_split=False, q_idx=6, kv_interleave=False, proj_deep=True)

# ---------------------------------------------------------------- constants
FULL_CFG = dict(
    R=512,        # queries per core
    W=2048,       # sliding window
    HID=1024,     # hidden size
    H=8,          # query heads
    KVH=2,        # kv heads
    D=128,        # head dim
    THETA=10000.0,
    NCORES=8,
)


def _derived(cfg):
    R, W, HID = cfg["R"], cfg["W"], cfg["HID"]
    KVW = W + R
    HC = HID // 128
    NKC = KVW // 128
    assert W == 2048 and R == 512 and HID == 1024
    return KVW, HC, NKC


def build_program(cfg):
    import concourse.bass as bass
    import concourse.tile as tile
    from concourse import bacc, mybir

    f32, f16, f8 = mybir.dt.float32, mybir.dt.float16, mybir.dt.float8e4
    DR = mybir.MatmulPerfMode.DoubleRow
    ts = bass.ts
    R, W, HID, H, KVH, D = (cfg["R"], cfg["W"], cfg["HID"], cfg["H"],
                            cfg["KVH"], cfg["D"])
    KVW, HC, NKC = _derived(cfg)
    NOB = W // 512            # old-region 512-blocks (4)
    NOC = W // 128            # old-region chunks (16)
    NRC = R // 128            # recent chunks (4)
    HC2 = HC // 2

    nc = bacc.Bacc("TRN2", target_bir_lowering=False, debug=False)

    def din(name, shape, dt):
        return nc.dram_tensor(name, shape, dt, kind="ExternalInput").ap()

    xT8 = din("xT8", [128 * HC * W], f8)          # 4 blocks of [128, HC, 512]
    xT16 = din("xT16", [128, HC * R], f16)        # own block [128, HC, 512]
    wq16 = din("wq16", [128, HC * H * D], f16)
    wk16 = din("wk16", [128, HC * KVH * D], f16)
    wv16 = din("wv16", [128, HC * KVH * D], f16)
    wk8 = din("wk8", [128, HC * KVH * D], f8)
    wv8 = din("wv8", [128, HC * KVH * D], f8)
    wo16 = din("wo16", [128, H * HID], f16)
    cosK = din("cosK", [128, KVW], f16)
    sinK = din("sinK", [128, KVW], f16)
    cosQ = din("cosQ", [128, R], f16)
    sinQ = din("sinQ", [128, R], f16)
    npadQ = din("npadQ", [1, R], f32)
    outT = nc.dram_tensor("outT", [HC, 128, R], f32, kind="ExternalOutput").ap()

    with tile.TileContext(nc) as tc, ExitStack() as ctx:
        const = ctx.enter_context(tc.tile_pool(name="const", bufs=1))
        kvp = ctx.enter_context(tc.tile_pool(name="kvp", bufs=1))
        work = ctx.enter_context(tc.tile_pool(name="work", bufs=2))
        psG = ctx.enter_context(tc.tile_pool(name="psG", bufs=2, space="PSUM"))
        psM = ctx.enter_context(tc.tile_pool(name="psM", bufs=1, space="PSUM"))
        psC = ctx.enter_context(tc.tile_pool(name="psC", bufs=2, space="PSUM"))
        psD = ctx.enter_context(tc.tile_pool(name="psD", bufs=1, space="PSUM"))

        # ---------------- input loads (recent/f16 path first)
        wk16_sb = const.tile([128, HC, KVH * D], f16)
        wv16_sb = const.tile([128, HC, KVH * D], f16)
        wq16_sb = const.tile([128, HC, H * D], f16)
        nc.sync.dma_start(out=wk16_sb, in_=wk16)
        nc.sync.dma_start(out=wv16_sb, in_=wv16)
        xT16_sb = const.tile([128, HC, R], f16)
        if TUNE["chunked_x16"]:
            for c in range(HC):
                nc.sync.dma_start(out=xT16_sb[:, c, :], in_=xT16[:, ts(c, R)])
        else:
            nc.sync.dma_start(out=xT16_sb, in_=xT16)
        nc.sync.dma_start(out=wq16_sb, in_=wq16)
        cosK_sb = const.tile([128, KVW], f16)
        sinK_sb = const.tile([128, KVW], f16)
        cosQ_sb = const.tile([128, R], f16)
        sinQ_sb = const.tile([128, R], f16)
        npad_sb = const.tile([1, R], f32)
        for dst, src in ((cosK_sb, cosK), (sinK_sb, sinK), (cosQ_sb, cosQ),
                         (sinQ_sb, sinQ), (npad_sb, npadQ)):
            nc.sync.dma_start(out=dst, in_=src)
        wk8_sb = const.tile([128, HC, KVH * D], f8)
        wv8_sb = const.tile([128, HC, KVH * D], f8)
        nc.sync.dma_start(out=wk8_sb, in_=wk8)
        nc.sync.dma_start(out=wv8_sb, in_=wv8)
        xT8_sb = const.tile([128, HC, W], f8)
        xt_last = None
        for b in range(NOB):
            src_ap = bass.AP(tensor=xT8.tensor, offset=b * 128 * HC * 512,
                             ap=[[HC * 512, 128], [512, HC], [1, 512]])
            xt_last = nc.sync.dma_start(out=xT8_sb[:, :, ts(b, 512)], in_=src_ap)
        wo16_sb = const.tile([128, H, HID], f16)
        di = nc.sync.dma_start(out=wo16_sb, in_=wo16)
        tile.add_dep_helper(di.ins, xt_last.ins, sync=True, reason="delay wo")

        ones16 = const.tile([128, 16], f16)
        nc.vector.memset(ones16, 1.0)
        ones8 = const.tile([128, 2, 16], f8)
        nc.vector.memset(ones8, 1.0)

        # ---------------- additive masks for edge chunks (compile-time)
        # scores^T chunk kc: keys jl = 128*kc + kp vs queries i (free).
        # valid iff i < jl <= i + W.
        masks = {}
        for kc in list(range(4)) + list(range(NOC, NKC)):
            m = const.tile([128, R], f16, name=f"mask{kc}")
            nc.gpsimd.memset(m, 1.0)
            if kc < 4:
                nc.gpsimd.affine_select(
                    out=m, in_=m, compare_op=mybir.AluOpType.is_ge, fill=0.0,
                    base=128 * kc - 1, pattern=[[-1, R]], channel_multiplier=1)
            else:
                nc.gpsimd.affine_select(
                    out=m, in_=m, compare_op=mybir.AluOpType.is_ge, fill=0.0,
                    base=W - 128 * kc, pattern=[[1, R]], channel_multiplier=-1)
            masks[kc] = m

        # ---------------- storage for K/V/ctx
        kT16_sb = [kvp.tile([128, R], f16, name=f"kT16_{g}") for g in range(KVH)]
        kT8_sb = [kvp.tile([64, 2, W], f8, name=f"kT8_{g}") for g in range(KVH)]
        v16_sb = [kvp.tile([128, NRC, 128], f16, name=f"v16_{g}") for g in range(KVH)]
        v8_sb = [kvp.tile([128, NOC, 128], f8, name=f"v8_{g}") for g in range(KVH)]
        ctxn = [kvp.tile([128, R], f16, name=f"ctxn{h}") for h in range(H)]

        proj_n = [0]

        def proj_ps():
            i = proj_n[0]
            proj_n[0] += 1
            n = 5 if TUNE["proj_deep"] else 4
            k = i % n
            if k == 0:
                return psM.tile([128, 512], f32, tag="mm", name=f"pp{i}")
            if k == 1:
                return psD.tile([128, 512], f32, tag="den", name=f"pp{i}")
            if k == 2 and TUNE["proj_deep"]:
                return psC.tile([128, 512], f32, tag="ctx", name=f"pp{i}")
            t = psG.tile([128, 2, 512], f32, tag="sc", name=f"pp{i}")
            return t[:, 0, :]

        # rope pieces: psum f32 -> f16 sbuf, rotate-half, cos/sin muls.
        # copy_eng: engine for the psum->sbuf copy.
        def rope_parts(src_ps, cos_ap, sin_ap, width, copy_eng, rot_eng=None):
            rot_eng = rot_eng or nc.vector
            sb = work.tile([128, width], f16, tag="ropesrc")
            copy_eng(out=sb, in_=src_ps)
            tmp = work.tile([128, width], f16, tag="rtmp")
            rot_eng.tensor_copy(out=tmp[0:64, :], in_=sb[64:128, :])
            rot_eng.tensor_copy(out=tmp[64:128, :], in_=sb[0:64, :])
            ta = work.tile([128, width], f16, tag="ra")
            nc.vector.tensor_mul(ta, sb, cos_ap)
            tb = work.tile([128, width], f16, tag="rb")
            nc.vector.tensor_mul(tb, tmp, sin_ap)
            return ta, tb

        # ---------------- K/V projections + RoPE (emitted as fine-grained
        # jobs so they interleave with attention steps of heads 0/1)
        def emit_kv_rec_K(g):
            kps = proj_ps()
            for c in range(HC):
                nc.tensor.matmul(kps, lhsT=wk16_sb[:, c, ts(g, D)],
                                 rhs=xT16_sb[:, c, :],
                                 start=(c == 0), stop=(c == HC - 1))
            ta, tb = rope_parts(kps, cosK_sb[:, W:KVW], sinK_sb[:, W:KVW],
                                R, nc.scalar.copy)
            nc.vector.tensor_add(kT16_sb[g], ta, tb)

        def emit_kv_rec_V(g):
            vps = proj_ps()
            for c in range(HC):
                nc.tensor.matmul(vps, lhsT=wv16_sb[:, c, ts(g, D)],
                                 rhs=xT16_sb[:, c, :],
                                 start=(c == 0), stop=(c == HC - 1))
            vT = work.tile([128, 512], f16, tag="vT")
            nc.scalar.copy(out=vT, in_=vps)
            nc.sync.dma_start_transpose(out=v16_sb[g], in_=vT)

        def emit_kv_old_K(g, b):
            kps = proj_ps()
            for c2 in range(HC2):
                nc.tensor.matmul(kps,
                                 lhsT=wk8_sb[:, 2 * c2:2 * c2 + 2, ts(g, D)],
                                 rhs=xT8_sb[:, 2 * c2:2 * c2 + 2, ts(b, 512)],
                                 start=(c2 == 0), stop=(c2 == HC2 - 1),
                                 perf_mode=DR)
            ta, tb = rope_parts(kps, cosK_sb[:, ts(b, 512)],
                                sinK_sb[:, ts(b, 512)], 512, nc.scalar.copy)
            nc.vector.tensor_add(kT8_sb[g][:, 0, ts(b, 512)],
                                 ta[0:64, :], tb[0:64, :])
            nc.vector.tensor_add(kT8_sb[g][:, 1, ts(b, 512)],
                                 ta[64:128, :], tb[64:128, :])

        def emit_kv_old_V(g, b):
            vps = proj_ps()
            for c2 in range(HC2):
                nc.tensor.matmul(vps,
                                 lhsT=wv8_sb[:, 2 * c2:2 * c2 + 2, ts(g, D)],
                                 rhs=xT8_sb[:, 2 * c2:2 * c2 + 2, ts(b, 512)],
                                 start=(c2 == 0), stop=(c2 == HC2 - 1),
                                 perf_mode=DR)
            vT = work.tile([128, 512], f16, tag="vT")
            nc.scalar.copy(out=vT, in_=vps)
            vtmp = work.tile([128, 4, 128], f16, tag="vtmp")
            nc.sync.dma_start_transpose(out=vtmp, in_=vT)
            nc.vector.tensor_copy(out=v8_sb[g][:, 4 * b:4 * b + 4, :],
                                  in_=vtmp)

        # ---------------- Q projection (fp16) -> qT16 + qT2 (fp8 d-paired)
        qts = {}

        def emit_q(h):
            qps = psM.tile([128, 512], f32, tag="mm", name=f"qps{h}")
            for c in range(HC):
                nc.tensor.matmul(qps, lhsT=wq16_sb[:, c, ts(h, D)],
                                 rhs=xT16_sb[:, c, :],
                                 start=(c == 0), stop=(c == HC - 1))
            ta, tb = rope_parts(qps, cosQ_sb, sinQ_sb, R, nc.vector.tensor_copy)
            qT16 = work.tile([128, R], f16, tag="qT16", name=f"qT16_{h}")
            nc.vector.tensor_add(qT16, ta, tb)
            qT2 = work.tile([64, 2, R], f8, tag="qT2", name=f"qT2_{h}")
            nc.vector.tensor_add(qT2[:, 0, :], ta[0:64, :], tb[0:64, :])
            nc.vector.tensor_add(qT2[:, 1, :], ta[64:128, :], tb[64:128, :])
            qts[h] = (qT16, qT2)

        # ---------------- attention per query head (scores pipelined one
        # step ahead of exp/PV so acts run back-to-back on ScalarE)
        def emit_attn(h):
            g = h // (H // KVH)
            qT16, qT2 = qts.pop(h)
            ctx_ps = psC.tile([128, R], f32, tag="ctx")
            den_ps = psD.tile([16, R], f32, tag="den")

            def sc_rec(pr):
                lo = 256 * pr
                scp = psG.tile([128, 2, 512], f32, tag="sc")
                for j in range(2):
                    r = 2 * pr + j
                    nc.tensor.matmul(scp[:, j, lo:], lhsT=kT16_sb[g][:, ts(r, 128)],
                                     rhs=qT16[:, lo:], start=True, stop=True)
                return scp

            def fin_rec(pr, scp):
                lo = 256 * pr
                P16 = work.tile([128, 2, 512], f16, tag="P16", bufs=3)
                nc.scalar.activation(out=P16[:, :, lo:], in_=scp[:, :, lo:],
                                     func=mybir.ActivationFunctionType.Exp)
                for j in range(2):
                    r = 2 * pr + j
                    c0 = max(lo, 128 * r)
                    nc.vector.tensor_mul(P16[:, j, c0:], P16[:, j, c0:],
                                         masks[NOC + r][:, c0:])
                for j in range(2):
                    r = 2 * pr + j
                    c0 = max(lo, 128 * r)
                    if TUNE["order_old_first"]:
                        first, last = False, (r == 3)
                    else:
                        first, last = (r == 0), False
                    nc.tensor.matmul(ctx_ps[:, c0:], lhsT=v16_sb[g][:, r, :],
                                     rhs=P16[:, j, c0:], start=first, stop=last,
                                     skip_group_check=True)
                    nc.tensor.matmul(den_ps[:, c0:], lhsT=ones16,
                                     rhs=P16[:, j, c0:], start=first, stop=last,
                                     skip_group_check=True)

            def sc_old(p):
                pw = 256 if p == 0 else 512
                scp = psG.tile([128, 2, 512], f32, tag="sc")
                for j in range(2):
                    kc = 2 * p + j
                    nc.tensor.matmul(scp[:, j, :pw],
                                     lhsT=kT8_sb[g][:, :, ts(kc, 128)],
                                     rhs=qT2[:, :, :pw], start=True, stop=True,
                                     perf_mode=DR)
                return scp

            def fin_old(p, scp):
                pw = 256 if p == 0 else 512
                P8 = work.tile([128, 2, 512], f8, tag="P8", bufs=3)
                nc.scalar.activation(out=P8[:, :, :pw], in_=scp[:, :, :pw],
                                     func=mybir.ActivationFunctionType.Exp)
                if p < 2:
                    for j in range(2):
                        kc = 2 * p + j
                        nc.vector.tensor_mul(P8[:, j, :pw], P8[:, j, :pw],
                                             masks[kc][:, :pw])
                if TUNE["order_old_first"]:
                    first, last = (p == 1), False
                else:
                    first, last = False, (p == 7)
                nc.tensor.matmul(ctx_ps[:, :pw], lhsT=v8_sb[g][:, 2 * p:2 * p + 2, :],
                                 rhs=P8[:, :, :pw], start=first, stop=last,
                                 perf_mode=DR, skip_group_check=True)
                nc.tensor.matmul(den_ps[:, :pw], lhsT=ones8,
                                 rhs=P8[:, :, :pw], start=first, stop=last,
                                 perf_mode=DR, skip_group_check=True)

            if TUNE["order_old_first"]:
                steps = ([(sc_old, fin_old, p) for p in (1, 0, 2, 3, 4, 5, 6, 7)] +
                         [(sc_rec, fin_rec, pr) for pr in range(2)])
            else:
                steps = ([(sc_rec, fin_rec, pr) for pr in range(2)] +
                         [(sc_old, fin_old, p) for p in range(8)])
            if TUNE["one_ahead"]:
                prev = None
                for idx, (scf, finf, arg) in enumerate(steps):
                    scp = scf(arg)
                    if prev is not None:
                        prev[0](prev[1], prev[2])
                    if idx == 6 and h + 1 < H:
                        emit_q(h + 1)
                    prev = (finf, arg, scp)
                prev[0](prev[1], prev[2])
            else:
                for idx, (scf, finf, arg) in enumerate(steps):
                    scp = scf(arg)
                    finf(arg, scp)
                    if idx == 6 and h + 1 < H:
                        emit_q(h + 1)

            # --- normalize
            drow = work.tile([1, R], f32, tag="drow")
            nc.vector.tensor_sub(drow, den_ps[0:1, :], npad_sb)
            rrow = work.tile([1, R], f32, tag="rrow")
            nc.vector.reciprocal_approx_fast(out=rrow, in_=drow)
            rbc = work.tile([128, R], f32, tag="rbc")
            nc.gpsimd.partition_broadcast(rbc, rrow)
            nc.vector.tensor_mul(ctxn[h], ctx_ps, rbc)

        emit_kv_rec_K(0)
        emit_kv_rec_V(0)
        emit_q(0)
        emit_kv_rec_K(1)
        emit_kv_rec_V(1)
        if TUNE["kv_interleave"]:
            for b in range(NOB):
                emit_kv_old_K(0, b)
                emit_kv_old_V(0, b)
                emit_kv_old_K(1, b)
                emit_kv_old_V(1, b)
        else:
            for g in range(KVH):
                for b in range(NOB):
                    emit_kv_old_K(g, b)
                    emit_kv_old_V(g, b)
        oacc = [kvp.tile([128, R], f32, name=f"oacc{ot}") for ot in range(HC)]

        def emit_oproj_half1(ots):
            for ot in ots:
                ops = psC.tile([128, R], f32, tag="ctx", name=f"oh{ot}")
                for hh in range(4):
                    nc.tensor.matmul(ops, lhsT=wo16_sb[:, hh, ts(ot, 128)],
                                     rhs=ctxn[hh], start=(hh == 0), stop=(hh == 3))
                nc.vector.tensor_copy(out=oacc[ot], in_=ops)

        for h in range(H):
            emit_attn(h)
            if TUNE["o_split"] and h >= 4:
                emit_oproj_half1(range(2 * (h - 4), 2 * (h - 4) + 2))

        # ---------------- o_proj tail
        for ot in range(HC):
            ops = psC.tile([128, R], f32, tag="ctx", name=f"ops{ot}")
            hs = range(4, H) if TUNE["o_split"] else range(H)
            first = hs[0] if hasattr(hs, '__getitem__') else 4
            hs = list(hs)
            for h in hs:
                nc.tensor.matmul(ops, lhsT=wo16_sb[:, h, ts(ot, 128)],
                                 rhs=ctxn[h], start=(h == hs[0]), stop=(h == hs[-1]))
            ob = work.tile([128, R], f32, tag="ob")
            if TUNE["o_split"]:
                nc.vector.tensor_add(ob, ops, oacc[ot])
            else:
                nc.vector.tensor_copy(out=ob, in_=ops)
            nc.sync.dma_start(out=outT[ot], in_=ob)

    nc.compile()
    return nc


# ---------------------------------------------------------------- host side
def host_prep(cfg, x, wq, wk, wv, wo, pos):
    """x: [S, HID] f32, weights as in reference, pos: [S] int. Returns list of
    per-core input dicts."""
    R, W, HID, H, KVH, D, TH = (cfg["R"], cfg["W"], cfg["HID"], cfg["H"],
                                cfg["KVH"], cfg["D"], cfg["THETA"])
    KVW, HC, NKC = _derived(cfg)
    S = x.shape[0]
    ncores = S // R
    f8 = ml_dtypes.float8_e4m3
    inv_freq = (1.0 / TH ** (np.arange(0, D, 2, dtype=np.float64) / D))

    def pack_pm(wt, ncol, dt):
        a = wt.reshape(-1, 128, ncol)            # [chunks, 128, ncol]
        return np.ascontiguousarray(
            a.transpose(1, 0, 2).reshape(128, -1)).astype(dt)

    wq16 = pack_pm(wq.T, H * D, np.float16)
    wk16 = pack_pm(wk.T, KVH * D, np.float16)
    wv16 = pack_pm(wv.T, KVH * D, np.float16)
    wk8 = pack_pm(wk.T, KVH * D, f8)
    wv8 = pack_pm(wv.T, KVH * D, f8)
    wo16 = pack_pm(wo.T, HID, np.float16)

    in_maps = []
    for c in range(ncores):
        lo, hi = c * R - W, c * R + R
        pad = max(0, -lo)
        xw = np.zeros((KVW, HID), np.float32)
        xw[pad:] = x[max(lo, 0):hi]
        xTa = xw.T.reshape(HC, 128, KVW)                  # [c, p, j]
        parts = []
        for b0 in range(0, W, 512):
            blk = xTa[:, :, b0:b0 + 512].transpose(1, 0, 2)   # [p, c, j]
            parts.append(np.ascontiguousarray(blk).astype(f8).reshape(-1))
        xT8 = np.concatenate(parts)
        xT16 = np.ascontiguousarray(
            xTa[:, :, W:KVW].transpose(1, 0, 2).reshape(128, -1)).astype(np.float16)

        pw = np.zeros(KVW, np.float64)
        pw[pad:] = pos[max(lo, 0):hi].astype(np.float64)
        ang = pw[:, None] * inv_freq[None, :]          # [KVW, 64]
        ck, sk = np.cos(ang).T, np.sin(ang).T          # [64, KVW]
        cosK32 = np.concatenate([ck, ck], 0).astype(np.float32)
        sinK32 = np.concatenate([-sk, sk], 0).astype(np.float32)
        scale = 1.0 / np.sqrt(D)
        cosQ = (cosK32[:, W:] * scale).astype(np.float16)
        sinQ = (sinK32[:, W:] * scale).astype(np.float16)
        cosK = cosK32.astype(np.float16)
        sinK = sinK32.astype(np.float16)
        i_idx = np.arange(R, dtype=np.float32)
        npad = np.maximum(0.0, pad - 1.0 - i_idx)[None, :].astype(np.float32)

        in_maps.append(dict(xT8=xT8, xT16=xT16, wq16=wq16, wk16=wk16,
                            wv16=wv16, wk8=wk8, wv8=wv8, wo16=wo16,
                            cosK=cosK, sinK=sinK, cosQ=cosQ, sinQ=sinQ,
                            npadQ=npad))
    return in_maps


def assemble(cfg, outs):
    """outs: list of per-core outT arrays [HC, 128, R] -> [S, HID] f32."""
    R, HID = cfg["R"], cfg["HID"]
    blocks = [o.transpose(2, 0, 1).reshape(R, HID) for o in outs]
    return np.concatenate(blocks, 0).astype(np.float32)


_PROGRAM_CACHE = {}


def kernel(hidden_states, wq, wk, wv, wo, position_ids):
    from concourse.bass_utils import run_bass_kernel_spmd

    cfg = FULL_CFG
    x = np.asarray(hidden_states, np.float32)
    assert x.ndim == 3 and x.shape[0] == 1
    x2 = x[0]
    pos = np.asarray(position_ids)[0]
    in_maps = host_prep(cfg, x2, np.asarray(wq, np.float32),
                        np.asarray(wk, np.float32), np.asarray(wv, np.float32),
                        np.asarray(wo, np.float32), pos)
    key = "full"
    if key not in _PROGRAM_CACHE:
        _PROGRAM_CACHE[key] = build_program(cfg)
    nc = _PROGRAM_CACHE[key]
    res = run_bass_kernel_spmd(nc, in_maps, list(range(cfg["NCORES"])))
    outs = [res.results[i]["outT"] for i in range(cfg["NCORES"])]
    out = assemble(cfg, outs)
    return out.reshape(1, *out.shape)


# revision 31
# speedup vs baseline: 1.0109x; 1.0109x over previous
"""Trainium2 Bass kernel for Mistral sliding-window attention (B=1, S=4096,
HID=1024, H=8 q-heads, KVH=2 kv-heads, D=128, WINDOW=2048).

Strategy: shard the 4096-token sequence across 8 NeuronCores (512 queries per
core). Each core recomputes K/V projections for its 2560-row key window
(own 512 rows + previous 2048), applies RoPE, computes sliding-window causal
attention for all 8 heads in the [keys, queries] orientation (scores^T), and
runs the full o_proj for its 512 rows. No collectives; the host concatenates
the per-core 512-row output blocks.

Precision plan (validated in numpy + HW smoke tests):
- "recent" keys (the core's own 512 rows, chunks 16..19) stay in fp16
  end-to-end: this region dominates accuracy for short-window (early)
  queries where softmax is concentrated.
- "old" keys (previous 2048 rows, chunks 0..15) run in fp8-e4m3 with
  DoubleRow matmuls (2 rows/cycle):
    * K/V projections from fp8 x / fp8 weights, pairing hidden chunks,
    * scores with d packed [64 partitions x 2 pairs],
    * exp P written directly as fp8 by the activation engine,
    * PV and the softmax-denominator ones-matmul pair CHUNKS via the
      [128, 2, 512] P layout -> one pass per two chunks (4x cheaper).
- Q and O projections fp16.

Softmax without max-subtraction (scores are O(1)); the denominator comes
from ones-matmuls, corrected for zero-padded keys via a host-provided
count (padded keys contribute exp(0)=1 exactly, even in fp8).
"""

import sys
import numpy as np
from contextlib import ExitStack

if "/opt/trn_rl_repo" not in sys.path:
    sys.path.insert(0, "/opt/trn_rl_repo")

import ml_dtypes

# tuning knobs resolved at build time (A/B'd via TimelineSim)
TUNE = dict(order_old_first=False, one_ahead=True, chunked_x16=True, o_split=False, q_idx=6, kv_interleave=False, proj_deep=False)

# ---------------------------------------------------------------- constants
FULL_CFG = dict(
    R=512,        # queries per core
    W=2048,       # sliding window
    HID=1024,     # hidden size
    H=8,          # query heads
    KVH=2,        # kv heads
    D=128,        # head dim
    THETA=10000.0,
    NCORES=8,
)


def _derived(cfg):
    R, W, HID = cfg["R"], cfg["W"], cfg["HID"]
    KVW = W + R
    HC = HID // 128
    NKC = KVW // 128
    assert W == 2048 and R == 512 and HID == 1024
    return KVW, HC, NKC


def build_program(cfg):
    import concourse.bass as bass
    import concourse.tile as tile
    from concourse import bacc, mybir

    f32, f16, f8 = mybir.dt.float32, mybir.dt.float16, mybir.dt.float8e4
    DR = mybir.MatmulPerfMode.DoubleRow
    ts = bass.ts
    R, W, HID, H, KVH, D = (cfg["R"], cfg["W"], cfg["HID"], cfg["H"],
                            cfg["KVH"], cfg["D"])
    KVW, HC, NKC = _derived(cfg)
    NOB = W // 512            # old-region 512-blocks (4)
    NOC = W // 128            # old-region chunks (16)
    NRC = R // 128            # recent chunks (4)
    HC2 = HC // 2

    nc = bacc.Bacc("TRN2", target_bir_lowering=False, debug=False)

    def din(name, shape, dt):
        return nc.dram_tensor(name, shape, dt, kind="ExternalInput").ap()

    xT8 = din("xT8", [128 * HC * W], f8)          # 4 blocks of [128, HC, 512]
    xT16 = din("xT16", [128, HC * R], f16)        # own block [128, HC, 512]
    wq16 = din("wq16", [128, HC * H * D], f16)
    wk16 = din("wk16", [128, HC * KVH * D], f16)
    wv16 = din("wv16", [128, HC * KVH * D], f16)
    wk8 = din("wk8", [128, HC * KVH * D], f8)
    wv8 = din("wv8", [128, HC * KVH * D], f8)
    wo16 = din("wo16", [128, H * HID], f16)
    cosK = din("cosK", [128, KVW], f16)
    sinK = din("sinK", [128, KVW], f16)
    cosQ = din("cosQ", [128, R], f16)
    sinQ = din("sinQ", [128, R], f16)
    npadQ = din("npadQ", [1, R], f32)
    outT = nc.dram_tensor("outT", [HC, 128, R], f32, kind="ExternalOutput").ap()

    with tile.TileContext(nc) as tc, ExitStack() as ctx:
        const = ctx.enter_context(tc.tile_pool(name="const", bufs=1))
        kvp = ctx.enter_context(tc.tile_pool(name="kvp", bufs=1))
        work = ctx.enter_context(tc.tile_pool(name="work", bufs=2))
        psG = ctx.enter_context(tc.tile_pool(name="psG", bufs=2, space="PSUM"))
        psM = ctx.enter_context(tc.tile_pool(name="psM", bufs=1, space="PSUM"))
        psC = ctx.enter_context(tc.tile_pool(name="psC", bufs=2, space="PSUM"))
        psD = ctx.enter_context(tc.tile_pool(name="psD", bufs=1, space="PSUM"))

        # ---------------- input loads (recent/f16 path first)
        wk16_sb = const.tile([128, HC, KVH * D], f16)
        wv16_sb = const.tile([128, HC, KVH * D], f16)
        wq16_sb = const.tile([128, HC, H * D], f16)
        nc.sync.dma_start(out=wk16_sb, in_=wk16)
        nc.sync.dma_start(out=wv16_sb, in_=wv16)
        xT16_sb = const.tile([128, HC, R], f16)
        if TUNE["chunked_x16"]:
            for c in range(HC):
                nc.sync.dma_start(out=xT16_sb[:, c, :], in_=xT16[:, ts(c, R)])
        else:
            nc.sync.dma_start(out=xT16_sb, in_=xT16)
        nc.sync.dma_start(out=wq16_sb, in_=wq16)
        cosK_sb = const.tile([128, KVW], f16)
        sinK_sb = const.tile([128, KVW], f16)
        cosQ_sb = const.tile([128, R], f16)
        sinQ_sb = const.tile([128, R], f16)
        npad_sb = const.tile([1, R], f32)
        for dst, src in ((cosK_sb, cosK), (sinK_sb, sinK), (cosQ_sb, cosQ),
                         (sinQ_sb, sinQ), (npad_sb, npadQ)):
            nc.sync.dma_start(out=dst, in_=src)
        wk8_sb = const.tile([128, HC, KVH * D], f8)
        wv8_sb = const.tile([128, HC, KVH * D], f8)
        nc.sync.dma_start(out=wk8_sb, in_=wk8)
        nc.sync.dma_start(out=wv8_sb, in_=wv8)
        xT8_sb = const.tile([128, HC, W], f8)
        xt_last = None
        for b in range(NOB):
            src_ap = bass.AP(tensor=xT8.tensor, offset=b * 128 * HC * 512,
                             ap=[[HC * 512, 128], [512, HC], [1, 512]])
            xt_last = nc.sync.dma_start(out=xT8_sb[:, :, ts(b, 512)], in_=src_ap)
        wo16_sb = const.tile([128, H, HID], f16)
        di = nc.sync.dma_start(out=wo16_sb, in_=wo16)
        tile.add_dep_helper(di.ins, xt_last.ins, sync=True, reason="delay wo")

        ones16 = const.tile([128, 16], f16)
        nc.vector.memset(ones16, 1.0)
        ones8 = const.tile([128, 2, 16], f8)
        nc.vector.memset(ones8, 1.0)

        # ---------------- additive masks for edge chunks (compile-time)
        # scores^T chunk kc: keys jl = 128*kc + kp vs queries i (free).
        # valid iff i < jl <= i + W.
        masks = {}
        for kc in list(range(4)) + list(range(NOC, NKC)):
            m = const.tile([128, R], f16, name=f"mask{kc}")
            nc.gpsimd.memset(m, 1.0)
            if kc < 4:
                nc.gpsimd.affine_select(
                    out=m, in_=m, compare_op=mybir.AluOpType.is_ge, fill=0.0,
                    base=128 * kc - 1, pattern=[[-1, R]], channel_multiplier=1)
            else:
                nc.gpsimd.affine_select(
                    out=m, in_=m, compare_op=mybir.AluOpType.is_ge, fill=0.0,
                    base=W - 128 * kc, pattern=[[1, R]], channel_multiplier=-1)
            masks[kc] = m

        # ---------------- storage for K/V/ctx
        kT16_sb = [kvp.tile([128, R], f16, name=f"kT16_{g}") for g in range(KVH)]
        kT8_sb = [kvp.tile([64, 2, W], f8, name=f"kT8_{g}") for g in range(KVH)]
        v16_sb = [kvp.tile([128, NRC, 128], f16, name=f"v16_{g}") for g in range(KVH)]
        v8_sb = [kvp.tile([128, NOC, 128], f8, name=f"v8_{g}") for g in range(KVH)]
        ctxn = [kvp.tile([128, R], f16, name=f"ctxn{h}") for h in range(H)]

        proj_n = [0]

        def proj_ps():
            i = proj_n[0]
            proj_n[0] += 1
            n = 5 if TUNE["proj_deep"] else 4
            k = i % n
            if k == 0:
                return psM.tile([128, 512], f32, tag="mm", name=f"pp{i}")
            if k == 1:
                return psD.tile([128, 512], f32, tag="den", name=f"pp{i}")
            if k == 2 and TUNE["proj_deep"]:
                return psC.tile([128, 512], f32, tag="ctx", name=f"pp{i}")
            t = psG.tile([128, 2, 512], f32, tag="sc", name=f"pp{i}")
            return t[:, 0, :]

        # rope pieces: psum f32 -> f16 sbuf, rotate-half, cos/sin muls.
        # copy_eng: engine for the psum->sbuf copy.
        def rope_parts(src_ps, cos_ap, sin_ap, width, copy_eng, rot_eng=None):
            rot_eng = rot_eng or nc.vector
            sb = work.tile([128, width], f16, tag="ropesrc")
            copy_eng(out=sb, in_=src_ps)
            tmp = work.tile([128, width], f16, tag="rtmp")
            rot_eng.tensor_copy(out=tmp[0:64, :], in_=sb[64:128, :])
            rot_eng.tensor_copy(out=tmp[64:128, :], in_=sb[0:64, :])
            ta = work.tile([128, width], f16, tag="ra")
            nc.vector.tensor_mul(ta, sb, cos_ap)
            tb = work.tile([128, width], f16, tag="rb")
            nc.vector.tensor_mul(tb, tmp, sin_ap)
            return ta, tb

        # ---------------- K/V projections + RoPE (emitted as fine-grained
        # jobs so they interleave with attention steps of heads 0/1)
        def emit_kv_rec_K(g):
            kps = proj_ps()
            for c in range(HC):
                nc.tensor.matmul(kps, lhsT=wk16_sb[:, c, ts(g, D)],
                                 rhs=xT16_sb[:, c, :],
                                 start=(c == 0), stop=(c == HC - 1))
            ta, tb = rope_parts(kps, cosK_sb[:, W:KVW], sinK_sb[:, W:KVW],
                                R, nc.scalar.copy)
            nc.vector.tensor_add(kT16_sb[g], ta, tb)

        def emit_kv_rec_V(g):
            vps = proj_ps()
            for c in range(HC):
                nc.tensor.matmul(vps, lhsT=wv16_sb[:, c, ts(g, D)],
                                 rhs=xT16_sb[:, c, :],
                                 start=(c == 0), stop=(c == HC - 1))
            vT = work.tile([128, 512], f16, tag="vT")
            nc.scalar.copy(out=vT, in_=vps)
            nc.sync.dma_start_transpose(out=v16_sb[g], in_=vT)

        def emit_kv_old_K(g, b):
            kps = proj_ps()
            for c2 in range(HC2):
                nc.tensor.matmul(kps,
                                 lhsT=wk8_sb[:, 2 * c2:2 * c2 + 2, ts(g, D)],
                                 rhs=xT8_sb[:, 2 * c2:2 * c2 + 2, ts(b, 512)],
                                 start=(c2 == 0), stop=(c2 == HC2 - 1),
                                 perf_mode=DR)
            ta, tb = rope_parts(kps, cosK_sb[:, ts(b, 512)],
                                sinK_sb[:, ts(b, 512)], 512, nc.scalar.copy)
            nc.vector.tensor_add(kT8_sb[g][:, 0, ts(b, 512)],
                                 ta[0:64, :], tb[0:64, :])
            nc.vector.tensor_add(kT8_sb[g][:, 1, ts(b, 512)],
                                 ta[64:128, :], tb[64:128, :])

        def emit_kv_old_V(g, b):
            vps = proj_ps()
            for c2 in range(HC2):
                nc.tensor.matmul(vps,
                                 lhsT=wv8_sb[:, 2 * c2:2 * c2 + 2, ts(g, D)],
                                 rhs=xT8_sb[:, 2 * c2:2 * c2 + 2, ts(b, 512)],
                                 start=(c2 == 0), stop=(c2 == HC2 - 1),
                                 perf_mode=DR)
            vT = work.tile([128, 512], f16, tag="vT")
            nc.scalar.copy(out=vT, in_=vps)
            vtmp = work.tile([128, 4, 128], f16, tag="vtmp")
            nc.sync.dma_start_transpose(out=vtmp, in_=vT)
            nc.vector.tensor_copy(out=v8_sb[g][:, 4 * b:4 * b + 4, :],
                                  in_=vtmp)

        # ---------------- Q projection (fp16) -> qT16 + qT2 (fp8 d-paired)
        qts = {}

        def emit_q(h):
            qps = psM.tile([128, 512], f32, tag="mm", name=f"qps{h}")
            for c in range(HC):
                nc.tensor.matmul(qps, lhsT=wq16_sb[:, c, ts(h, D)],
                                 rhs=xT16_sb[:, c, :],
                                 start=(c == 0), stop=(c == HC - 1))
            ta, tb = rope_parts(qps, cosQ_sb, sinQ_sb, R, nc.vector.tensor_copy)
            qT16 = work.tile([128, R], f16, tag="qT16", name=f"qT16_{h}")
            nc.vector.tensor_add(qT16, ta, tb)
            qT2 = work.tile([64, 2, R], f8, tag="qT2", name=f"qT2_{h}")
            nc.vector.tensor_add(qT2[:, 0, :], ta[0:64, :], tb[0:64, :])
            nc.vector.tensor_add(qT2[:, 1, :], ta[64:128, :], tb[64:128, :])
            qts[h] = (qT16, qT2)

        # ---------------- attention per query head (scores pipelined one
        # step ahead of exp/PV so acts run back-to-back on ScalarE)
        def emit_attn(h):
            g = h // (H // KVH)
            qT16, qT2 = qts.pop(h)
            ctx_ps = psC.tile([128, R], f32, tag="ctx")
            den_ps = psD.tile([16, R], f32, tag="den")

            def sc_rec(pr):
                lo = 256 * pr
                scp = psG.tile([128, 2, 512], f32, tag="sc")
                for j in range(2):
                    r = 2 * pr + j
                    nc.tensor.matmul(scp[:, j, lo:], lhsT=kT16_sb[g][:, ts(r, 128)],
                                     rhs=qT16[:, lo:], start=True, stop=True)
                return scp

            def fin_rec(pr, scp):
                lo = 256 * pr
                P16 = work.tile([128, 2, 512], f16, tag="P16", bufs=3)
                nc.scalar.activation(out=P16[:, :, lo:], in_=scp[:, :, lo:],
                                     func=mybir.ActivationFunctionType.Exp)
                for j in range(2):
                    r = 2 * pr + j
                    c0 = max(lo, 128 * r)
                    nc.vector.tensor_mul(P16[:, j, c0:], P16[:, j, c0:],
                                         masks[NOC + r][:, c0:])
                for j in range(2):
                    r = 2 * pr + j
                    c0 = max(lo, 128 * r)
                    if TUNE["order_old_first"]:
                        first, last = False, (r == 3)
                    else:
                        first, last = (r == 0), False
                    nc.tensor.matmul(ctx_ps[:, c0:], lhsT=v16_sb[g][:, r, :],
                                     rhs=P16[:, j, c0:], start=first, stop=last,
                                     skip_group_check=True)
                    nc.tensor.matmul(den_ps[:, c0:], lhsT=ones16,
                                     rhs=P16[:, j, c0:], start=first, stop=last,
                                     skip_group_check=True)

            def sc_old(p):
                pw = 256 if p == 0 else 512
                scp = psG.tile([128, 2, 512], f32, tag="sc")
                for j in range(2):
                    kc = 2 * p + j
                    nc.tensor.matmul(scp[:, j, :pw],
                                     lhsT=kT8_sb[g][:, :, ts(kc, 128)],
                                     rhs=qT2[:, :, :pw], start=True, stop=True,
                                     perf_mode=DR)
                return scp

            def fin_old(p, scp):
                pw = 256 if p == 0 else 512
                P8 = work.tile([128, 2, 512], f8, tag="P8", bufs=3)
                nc.scalar.activation(out=P8[:, :, :pw], in_=scp[:, :, :pw],
                                     func=mybir.ActivationFunctionType.Exp)
                if p < 2:
                    for j in range(2):
                        kc = 2 * p + j
                        nc.vector.tensor_mul(P8[:, j, :pw], P8[:, j, :pw],
                                             masks[kc][:, :pw])
                if TUNE["order_old_first"]:
                    first, last = (p == 1), False
                else:
                    first, last = False, (p == 7)
                nc.tensor.matmul(ctx_ps[:, :pw], lhsT=v8_sb[g][:, 2 * p:2 * p + 2, :],
                                 rhs=P8[:, :, :pw], start=first, stop=last,
                                 perf_mode=DR, skip_group_check=True)
                nc.tensor.matmul(den_ps[:, :pw], lhsT=ones8,
                                 rhs=P8[:, :, :pw], start=first, stop=last,
                                 perf_mode=DR, skip_group_check=True)

            if TUNE["order_old_first"]:
                steps = ([(sc_old, fin_old, p) for p in (1, 0, 2, 3, 4, 5, 6, 7)] +
                         [(sc_rec, fin_rec, pr) for pr in range(2)])
            else:
                steps = ([(sc_rec, fin_rec, pr) for pr in range(2)] +
                         [(sc_old, fin_old, p) for p in range(8)])
            if TUNE["one_ahead"]:
                prev = None
                for idx, (scf, finf, arg) in enumerate(steps):
                    scp = scf(arg)
                    if prev is not None:
                        prev[0](prev[1], prev[2])
                    if idx == 6 and h + 1 < H:
                        emit_q(h + 1)
                    prev = (finf, arg, scp)
                prev[0](prev[1], prev[2])
            else:
                for idx, (scf, finf, arg) in enumerate(steps):
                    scp = scf(arg)
                    finf(arg, scp)
                    if idx == 6 and h + 1 < H:
                        emit_q(h + 1)

            # --- normalize
            drow = work.tile([1, R], f32, tag="drow")
            nc.vector.tensor_sub(drow, den_ps[0:1, :], npad_sb)
            rrow = work.tile([1, R], f32, tag="rrow")
            nc.vector.reciprocal_approx_fast(out=rrow, in_=drow)
            rbc = work.tile([128, R], f32, tag="rbc")
            nc.gpsimd.partition_broadcast(rbc, rrow)
            nc.vector.tensor_mul(ctxn[h], ctx_ps, rbc)

        emit_kv_rec_K(0)
        emit_kv_rec_V(0)
        emit_q(0)
        emit_kv_rec_K(1)
        emit_kv_rec_V(1)
        if TUNE["kv_interleave"]:
            for b in range(NOB):
                emit_kv_old_K(0, b)
                emit_kv_old_V(0, b)
                emit_kv_old_K(1, b)
                emit_kv_old_V(1, b)
        else:
            for g in range(KVH):
                for b in range(NOB):
                    emit_kv_old_K(g, b)
                    emit_kv_old_V(g, b)
        oacc = [kvp.tile([128, R], f32, name=f"oacc{ot}") for ot in range(HC)]

        def emit_oproj_half1(ots):
            for ot in ots:
                ops = psC.tile([128, R], f32, tag="ctx", name=f"oh{ot}")
                for hh in range(4):
                    nc.tensor.matmul(ops, lhsT=wo16_sb[:, hh, ts(ot, 128)],
                                     rhs=ctxn[hh], start=(hh == 0), stop=(hh == 3))
                nc.vector.tensor_copy(out=oacc[ot], in_=ops)

        for h in range(H):
            emit_attn(h)
            if TUNE["o_split"] and h >= 4:
                emit_oproj_half1(range(2 * (h - 4), 2 * (h - 4) + 2))

        # ---------------- o_proj tail
        for ot in range(HC):
            ops = psC.tile([128, R], f32, tag="ctx", name=f"ops{ot}")
            hs = range(4, H) if TUNE["o_split"] else range(H)
            first = hs[0] if hasattr(hs, '__getitem__') else 4
            hs = list(hs)
            for h in hs:
                nc.tensor.matmul(ops, lhsT=wo16_sb[:, h, ts(ot, 128)],
                                 rhs=ctxn[h], start=(h == hs[0]), stop=(h == hs[-1]))
            ob = work.tile([128, R], f32, tag="ob")
            if TUNE["o_split"]:
                nc.vector.tensor_add(ob, ops, oacc[ot])
            else:
                nc.vector.tensor_copy(out=ob, in_=ops)
            nc.sync.dma_start(out=outT[ot], in_=ob)

    nc.compile()
    return nc


# ---------------------------------------------------------------- host side
def host_prep(cfg, x, wq, wk, wv, wo, pos):
    """x: [S, HID] f32, weights as in reference, pos: [S] int. Returns list of
    per-core input dicts."""
    R, W, HID, H, KVH, D, TH = (cfg["R"], cfg["W"], cfg["HID"], cfg["H"],
                                cfg["KVH"], cfg["D"], cfg["THETA"])
    KVW, HC, NKC = _derived(cfg)
    S = x.shape[0]
    ncores = S // R
    f8 = ml_dtypes.float8_e4m3
    inv_freq = (1.0 / TH ** (np.arange(0, D, 2, dtype=np.float64) / D))

    def pack_pm(wt, ncol, dt):
        a = wt.reshape(-1, 128, ncol)            # [chunks, 128, ncol]
        return np.ascontiguousarray(
            a.transpose(1, 0, 2).reshape(128, -1)).astype(dt)

    wq16 = pack_pm(wq.T, H * D, np.float16)
    wk16 = pack_pm(wk.T, KVH * D, np.float16)
    wv16 = pack_pm(wv.T, KVH * D, np.float16)
    wk8 = pack_pm(wk.T, KVH * D, f8)
    wv8 = pack_pm(wv.T, KVH * D, f8)
    wo16 = pack_pm(wo.T, HID, np.float16)

    in_maps = []
    for c in range(ncores):
        lo, hi = c * R - W, c * R + R
        pad = max(0, -lo)
        xw = np.zeros((KVW, HID), np.float32)
        xw[pad:] = x[max(lo, 0):hi]
        xTa = xw.T.reshape(HC, 128, KVW)                  # [c, p, j]
        parts = []
        for b0 in range(0, W, 512):
            blk = xTa[:, :, b0:b0 + 512].transpose(1, 0, 2)   # [p, c, j]
            parts.append(np.ascontiguousarray(blk).astype(f8).reshape(-1))
        xT8 = np.concatenate(parts)
        xT16 = np.ascontiguousarray(
            xTa[:, :, W:KVW].transpose(1, 0, 2).reshape(128, -1)).astype(np.float16)

        pw = np.zeros(KVW, np.float64)
        pw[pad:] = pos[max(lo, 0):hi].astype(np.float64)
        ang = pw[:, None] * inv_freq[None, :]          # [KVW, 64]
        ck, sk = np.cos(ang).T, np.sin(ang).T          # [64, KVW]
        cosK32 = np.concatenate([ck, ck], 0).astype(np.float32)
        sinK32 = np.concatenate([-sk, sk], 0).astype(np.float32)
        scale = 1.0 / np.sqrt(D)
        cosQ = (cosK32[:, W:] * scale).astype(np.float16)
        sinQ = (sinK32[:, W:] * scale).astype(np.float16)
        cosK = cosK32.astype(np.float16)
        sinK = sinK32.astype(np.float16)
        i_idx = np.arange(R, dtype=np.float32)
        npad = np.maximum(0.0, pad - 1.0 - i_idx)[None, :].astype(np.float32)

        in_maps.append(dict(xT8=xT8, xT16=xT16, wq16=wq16, wk16=wk16,
                            wv16=wv16, wk8=wk8, wv8=wv8, wo16=wo16,
                            cosK=cosK, sinK=sinK, cosQ=cosQ, sinQ=sinQ,
                            npadQ=npad))
    return in_maps


def assemble(cfg, outs):
    """outs: list of per-core outT arrays [HC, 128, R] -> [S, HID] f32."""
    R, HID = cfg["R"], cfg["HID"]
    blocks = [o.transpose(2, 0, 1).reshape(R, HID) for o in outs]
    return np.concatenate(blocks, 0).astype(np.float32)


_PROGRAM_CACHE = {}


def kernel(hidden_states, wq, wk, wv, wo, position_ids):
    from concourse.bass_utils import run_bass_kernel_spmd

    cfg = FULL_CFG
    x = np.asarray(hidden_states, np.float32)
    assert x.ndim == 3 and x.shape[0] == 1
    x2 = x[0]
    pos = np.asarray(position_ids)[0]
    in_maps = host_prep(cfg, x2, np.asarray(wq, np.float32),
                        np.asarray(wk, np.float32), np.asarray(wv, np.float32),
                        np.asarray(wo, np.float32), pos)
    key = "full"
    if key not in _PROGRAM_CACHE:
        _PROGRAM_CACHE[key] = build_program(cfg)
    nc = _PROGRAM_CACHE[key]
    res = run_bass_kernel_spmd(nc, in_maps, list(range(cfg["NCORES"])))
    outs = [res.results[i]["outT"] for i in range(cfg["NCORES"])]
    out = assemble(cfg, outs)
    return out.reshape(1, *out.shape)


# revision 36
# speedup vs baseline: 1.0359x; 1.0247x over previous
"""Trainium2 Bass kernel for Mistral sliding-window attention (B=1, S=4096,
HID=1024, H=8 q-heads, KVH=2 kv-heads, D=128, WINDOW=2048).

Strategy: shard the 4096-token sequence across 8 NeuronCores (512 queries per
core). Each core recomputes K/V projections for its 2560-row key window
(own 512 rows + previous 2048), applies RoPE, computes sliding-window causal
attention for all 8 heads in the [keys, queries] orientation (scores^T), and
runs the full o_proj for its 512 rows. No collectives; the host concatenates
the per-core 512-row output blocks.

Precision plan (validated in numpy + HW smoke tests):
- "recent" keys (the core's own 512 rows, chunks 16..19) stay in fp16
  end-to-end: this region dominates accuracy for short-window (early)
  queries where softmax is concentrated.
- "old" keys (previous 2048 rows, chunks 0..15) run in fp8-e4m3 with
  DoubleRow matmuls (2 rows/cycle):
    * K/V projections from fp8 x / fp8 weights, pairing hidden chunks,
    * scores with d packed [64 partitions x 2 pairs],
    * exp P written directly as fp8 by the activation engine,
    * PV and the softmax-denominator ones-matmul pair CHUNKS via the
      [128, 2, 512] P layout -> one pass per two chunks (4x cheaper).
- Q and O projections fp16.

Softmax without max-subtraction (scores are O(1)); the denominator comes
from ones-matmuls, corrected for zero-padded keys via a host-provided
count (padded keys contribute exp(0)=1 exactly, even in fp8).
"""

import sys
import numpy as np
from contextlib import ExitStack

if "/opt/trn_rl_repo" not in sys.path:
    sys.path.insert(0, "/opt/trn_rl_repo")

import ml_dtypes

# tuning knobs resolved at build time (A/B'd via TimelineSim)
TUNE = dict(order_old_first=False, one_ahead=True, chunked_x16=True, o_split=False, q_idx=6, kv_jobs=True, proj_deep=False)

# ---------------------------------------------------------------- constants
FULL_CFG = dict(
    R=512,        # queries per core
    W=2048,       # sliding window
    HID=1024,     # hidden size
    H=8,          # query heads
    KVH=2,        # kv heads
    D=128,        # head dim
    THETA=10000.0,
    NCORES=8,
)


def _derived(cfg):
    R, W, HID = cfg["R"], cfg["W"], cfg["HID"]
    KVW = W + R
    HC = HID // 128
    NKC = KVW // 128
    assert W == 2048 and R == 512 and HID == 1024
    return KVW, HC, NKC


def build_program(cfg):
    import concourse.bass as bass
    import concourse.tile as tile
    from concourse import bacc, mybir

    f32, f16, f8 = mybir.dt.float32, mybir.dt.float16, mybir.dt.float8e4
    DR = mybir.MatmulPerfMode.DoubleRow
    ts = bass.ts
    R, W, HID, H, KVH, D = (cfg["R"], cfg["W"], cfg["HID"], cfg["H"],
                            cfg["KVH"], cfg["D"])
    KVW, HC, NKC = _derived(cfg)
    NOB = W // 512            # old-region 512-blocks (4)
    NOC = W // 128            # old-region chunks (16)
    NRC = R // 128            # recent chunks (4)
    HC2 = HC // 2

    nc = bacc.Bacc("TRN2", target_bir_lowering=False, debug=False)

    def din(name, shape, dt):
        return nc.dram_tensor(name, shape, dt, kind="ExternalInput").ap()

    xT8 = din("xT8", [128 * HC * W], f8)          # 4 blocks of [128, HC, 512]
    xT16 = din("xT16", [128, HC * R], f16)        # own block [128, HC, 512]
    wq16 = din("wq16", [128, HC * H * D], f16)
    wk16 = din("wk16", [128, HC * KVH * D], f16)
    wv16 = din("wv16", [128, HC * KVH * D], f16)
    wk8 = din("wk8", [128, HC * KVH * D], f8)
    wv8 = din("wv8", [128, HC * KVH * D], f8)
    wo16 = din("wo16", [128, H * HID], f16)
    cosK = din("cosK", [128, KVW], f16)
    sinK = din("sinK", [128, KVW], f16)
    cosQ = din("cosQ", [128, R], f16)
    sinQ = din("sinQ", [128, R], f16)
    npadQ = din("npadQ", [1, R], f32)
    outT = nc.dram_tensor("outT", [HC, 128, R], f32, kind="ExternalOutput").ap()

    with tile.TileContext(nc) as tc, ExitStack() as ctx:
        const = ctx.enter_context(tc.tile_pool(name="const", bufs=1))
        kvp = ctx.enter_context(tc.tile_pool(name="kvp", bufs=1))
        work = ctx.enter_context(tc.tile_pool(name="work", bufs=2))
        psG = ctx.enter_context(tc.tile_pool(name="psG", bufs=2, space="PSUM"))
        psM = ctx.enter_context(tc.tile_pool(name="psM", bufs=1, space="PSUM"))
        psC = ctx.enter_context(tc.tile_pool(name="psC", bufs=2, space="PSUM"))
        psD = ctx.enter_context(tc.tile_pool(name="psD", bufs=1, space="PSUM"))

        # ---------------- input loads (recent/f16 path first)
        wk16_sb = const.tile([128, HC, KVH * D], f16)
        wv16_sb = const.tile([128, HC, KVH * D], f16)
        wq16_sb = const.tile([128, HC, H * D], f16)
        nc.sync.dma_start(out=wk16_sb, in_=wk16)
        nc.sync.dma_start(out=wv16_sb, in_=wv16)
        xT16_sb = const.tile([128, HC, R], f16)
        if TUNE["chunked_x16"]:
            for c in range(HC):
                nc.sync.dma_start(out=xT16_sb[:, c, :], in_=xT16[:, ts(c, R)])
        else:
            nc.sync.dma_start(out=xT16_sb, in_=xT16)
        nc.sync.dma_start(out=wq16_sb, in_=wq16)
        cosK_sb = const.tile([128, KVW], f16)
        sinK_sb = const.tile([128, KVW], f16)
        cosQ_sb = const.tile([128, R], f16)
        sinQ_sb = const.tile([128, R], f16)
        npad_sb = const.tile([1, R], f32)
        for dst, src in ((cosK_sb, cosK), (sinK_sb, sinK), (cosQ_sb, cosQ),
                         (sinQ_sb, sinQ), (npad_sb, npadQ)):
            nc.sync.dma_start(out=dst, in_=src)
        wk8_sb = const.tile([128, HC, KVH * D], f8)
        wv8_sb = const.tile([128, HC, KVH * D], f8)
        nc.sync.dma_start(out=wk8_sb, in_=wk8)
        nc.sync.dma_start(out=wv8_sb, in_=wv8)
        xT8_sb = const.tile([128, HC, W], f8)
        xt_last = None
        for b in range(NOB):
            src_ap = bass.AP(tensor=xT8.tensor, offset=b * 128 * HC * 512,
                             ap=[[HC * 512, 128], [512, HC], [1, 512]])
            xt_last = nc.sync.dma_start(out=xT8_sb[:, :, ts(b, 512)], in_=src_ap)
        wo16_sb = const.tile([128, H, HID], f16)
        di = nc.sync.dma_start(out=wo16_sb, in_=wo16)
        tile.add_dep_helper(di.ins, xt_last.ins, sync=True, reason="delay wo")

        ones16 = const.tile([128, 16], f16)
        nc.vector.memset(ones16, 1.0)
        ones8 = const.tile([128, 2, 16], f8)
        nc.vector.memset(ones8, 1.0)

        # ---------------- additive masks for edge chunks (compile-time)
        # scores^T chunk kc: keys jl = 128*kc + kp vs queries i (free).
        # valid iff i < jl <= i + W.
        masks = {}
        for kc in list(range(4)) + list(range(NOC, NKC)):
            m = const.tile([128, R], f16, name=f"mask{kc}")
            nc.gpsimd.memset(m, 1.0)
            if kc < 4:
                nc.gpsimd.affine_select(
                    out=m, in_=m, compare_op=mybir.AluOpType.is_ge, fill=0.0,
                    base=128 * kc - 1, pattern=[[-1, R]], channel_multiplier=1)
            else:
                nc.gpsimd.affine_select(
                    out=m, in_=m, compare_op=mybir.AluOpType.is_ge, fill=0.0,
                    base=W - 128 * kc, pattern=[[1, R]], channel_multiplier=-1)
            masks[kc] = m

        # ---------------- storage for K/V/ctx
        kT16_sb = [kvp.tile([128, R], f16, name=f"kT16_{g}") for g in range(KVH)]
        kT8_sb = [kvp.tile([64, 2, W], f8, name=f"kT8_{g}") for g in range(KVH)]
        v16_sb = [kvp.tile([128, NRC, 128], f16, name=f"v16_{g}") for g in range(KVH)]
        v8_sb = [kvp.tile([128, NOC, 128], f8, name=f"v8_{g}") for g in range(KVH)]
        ctxn = [kvp.tile([128, R], f16, name=f"ctxn{h}") for h in range(H)]

        proj_n = [0]
        in_attn = [False]

        def proj_ps():
            i = proj_n[0]
            proj_n[0] += 1
            if in_attn[0]:
                # den psum is live per-head now; rotate psM/psG only
                if i % 2 == 0:
                    return psM.tile([128, 512], f32, tag="mm", name=f"pp{i}")
                t = psG.tile([128, 2, 512], f32, tag="sc", name=f"pp{i}")
                return t[:, 0, :]
            k = i % 4
            if k == 0:
                return psM.tile([128, 512], f32, tag="mm", name=f"pp{i}")
            if k == 1:
                return psD.tile([128, 512], f32, tag="den", name=f"pp{i}")
            t = psG.tile([128, 2, 512], f32, tag="sc", name=f"pp{i}")
            return t[:, 0, :]

        # rope pieces: psum f32 -> f16 sbuf, rotate-half, cos/sin muls.
        # copy_eng: engine for the psum->sbuf copy.
        def rope_parts(src_ps, cos_ap, sin_ap, width, copy_eng, rot_eng=None):
            rot_eng = rot_eng or nc.vector
            sb = work.tile([128, width], f16, tag="ropesrc")
            copy_eng(out=sb, in_=src_ps)
            tmp = work.tile([128, width], f16, tag="rtmp")
            rot_eng.tensor_copy(out=tmp[0:64, :], in_=sb[64:128, :])
            rot_eng.tensor_copy(out=tmp[64:128, :], in_=sb[0:64, :])
            ta = work.tile([128, width], f16, tag="ra")
            nc.vector.tensor_mul(ta, sb, cos_ap)
            tb = work.tile([128, width], f16, tag="rb")
            nc.vector.tensor_mul(tb, tmp, sin_ap)
            return ta, tb

        # ---------------- K/V projections + RoPE (emitted as fine-grained
        # jobs so they interleave with attention steps of heads 0/1)
        def emit_kv_rec_K(g):
            kps = proj_ps()
            for c in range(HC):
                nc.tensor.matmul(kps, lhsT=wk16_sb[:, c, ts(g, D)],
                                 rhs=xT16_sb[:, c, :],
                                 start=(c == 0), stop=(c == HC - 1))
            ta, tb = rope_parts(kps, cosK_sb[:, W:KVW], sinK_sb[:, W:KVW],
                                R, nc.scalar.copy)
            nc.vector.tensor_add(kT16_sb[g], ta, tb)

        def emit_kv_rec_V(g):
            vps = proj_ps()
            for c in range(HC):
                nc.tensor.matmul(vps, lhsT=wv16_sb[:, c, ts(g, D)],
                                 rhs=xT16_sb[:, c, :],
                                 start=(c == 0), stop=(c == HC - 1))
            vT = work.tile([128, 512], f16, tag="vT")
            nc.scalar.copy(out=vT, in_=vps)
            nc.sync.dma_start_transpose(out=v16_sb[g], in_=vT)

        def emit_kv_old_K(g, b):
            kps = proj_ps()
            for c2 in range(HC2):
                nc.tensor.matmul(kps,
                                 lhsT=wk8_sb[:, 2 * c2:2 * c2 + 2, ts(g, D)],
                                 rhs=xT8_sb[:, 2 * c2:2 * c2 + 2, ts(b, 512)],
                                 start=(c2 == 0), stop=(c2 == HC2 - 1),
                                 perf_mode=DR)
            ta, tb = rope_parts(kps, cosK_sb[:, ts(b, 512)],
                                sinK_sb[:, ts(b, 512)], 512, nc.scalar.copy)
            nc.vector.tensor_add(kT8_sb[g][:, 0, ts(b, 512)],
                                 ta[0:64, :], tb[0:64, :])
            nc.vector.tensor_add(kT8_sb[g][:, 1, ts(b, 512)],
                                 ta[64:128, :], tb[64:128, :])

        def emit_kv_old_V(g, b):
            vps = proj_ps()
            for c2 in range(HC2):
                nc.tensor.matmul(vps,
                                 lhsT=wv8_sb[:, 2 * c2:2 * c2 + 2, ts(g, D)],
                                 rhs=xT8_sb[:, 2 * c2:2 * c2 + 2, ts(b, 512)],
                                 start=(c2 == 0), stop=(c2 == HC2 - 1),
                                 perf_mode=DR)
            vT = work.tile([128, 512], f16, tag="vT")
            nc.scalar.copy(out=vT, in_=vps)
            vtmp = work.tile([128, 4, 128], f16, tag="vtmp")
            nc.sync.dma_start_transpose(out=vtmp, in_=vT)
            nc.vector.tensor_copy(out=v8_sb[g][:, 4 * b:4 * b + 4, :],
                                  in_=vtmp)

        # ---------------- Q projection (fp16) -> qT16 + qT2 (fp8 d-paired)
        qts = {}

        def emit_q(h):
            qps = psM.tile([128, 512], f32, tag="mm", name=f"qps{h}")
            for c in range(HC):
                nc.tensor.matmul(qps, lhsT=wq16_sb[:, c, ts(h, D)],
                                 rhs=xT16_sb[:, c, :],
                                 start=(c == 0), stop=(c == HC - 1))
            ta, tb = rope_parts(qps, cosQ_sb, sinQ_sb, R, nc.vector.tensor_copy)
            qT16 = work.tile([128, R], f16, tag="qT16", name=f"qT16_{h}")
            nc.vector.tensor_add(qT16, ta, tb)
            qT2 = work.tile([64, 2, R], f8, tag="qT2", name=f"qT2_{h}")
            nc.vector.tensor_add(qT2[:, 0, :], ta[0:64, :], tb[0:64, :])
            nc.vector.tensor_add(qT2[:, 1, :], ta[64:128, :], tb[64:128, :])
            qts[h] = (qT16, qT2)

        # ---------------- filler jobs: group-1 K/V emitted sparsely during
        # heads 0-2 (g=1 data first needed by head 4)
        jobs = []

        def emit_job():
            if jobs:
                jobs.pop(0)()

        # ---------------- attention per query head (scores pipelined one
        # step ahead of exp/PV so acts run back-to-back on ScalarE)
        def emit_attn(h):
            g = h // (H // KVH)
            qT16, qT2 = qts.pop(h)
            ctx_ps = psC.tile([128, R], f32, tag="ctx")
            den_ps = psD.tile([16, R], f32, tag="den")

            def sc_rec(pr):
                lo = 256 * pr
                scp = psG.tile([128, 2, 512], f32, tag="sc")
                for j in range(2):
                    r = 2 * pr + j
                    nc.tensor.matmul(scp[:, j, lo:], lhsT=kT16_sb[g][:, ts(r, 128)],
                                     rhs=qT16[:, lo:], start=True, stop=True)
                return scp

            def fin_rec(pr, scp):
                lo = 256 * pr
                P16 = work.tile([128, 2, 512], f16, tag="P16", bufs=3)
                nc.scalar.activation(out=P16[:, :, lo:], in_=scp[:, :, lo:],
                                     func=mybir.ActivationFunctionType.Exp)
                for j in range(2):
                    r = 2 * pr + j
                    c0 = max(lo, 128 * r)
                    nc.vector.tensor_mul(P16[:, j, c0:], P16[:, j, c0:],
                                         masks[NOC + r][:, c0:])
                for j in range(2):
                    r = 2 * pr + j
                    c0 = max(lo, 128 * r)
                    if TUNE["order_old_first"]:
                        first, last = False, (r == 3)
                    else:
                        first, last = (r == 0), False
                    nc.tensor.matmul(ctx_ps[:, c0:], lhsT=v16_sb[g][:, r, :],
                                     rhs=P16[:, j, c0:], start=first, stop=last,
                                     skip_group_check=True)
                    nc.tensor.matmul(den_ps[:, c0:], lhsT=ones16,
                                     rhs=P16[:, j, c0:], start=first, stop=last,
                                     skip_group_check=True)

            def sc_old(p):
                pw = 256 if p == 0 else 512
                scp = psG.tile([128, 2, 512], f32, tag="sc")
                for j in range(2):
                    kc = 2 * p + j
                    nc.tensor.matmul(scp[:, j, :pw],
                                     lhsT=kT8_sb[g][:, :, ts(kc, 128)],
                                     rhs=qT2[:, :, :pw], start=True, stop=True,
                                     perf_mode=DR)
                return scp

            def fin_old(p, scp):
                pw = 256 if p == 0 else 512
                P8 = work.tile([128, 2, 512], f8, tag="P8", bufs=3)
                nc.scalar.activation(out=P8[:, :, :pw], in_=scp[:, :, :pw],
                                     func=mybir.ActivationFunctionType.Exp)
                if p < 2:
                    for j in range(2):
                        kc = 2 * p + j
                        nc.vector.tensor_mul(P8[:, j, :pw], P8[:, j, :pw],
                                             masks[kc][:, :pw])
                if TUNE["order_old_first"]:
                    first, last = (p == 1), False
                else:
                    first, last = False, (p == 7)
                nc.tensor.matmul(ctx_ps[:, :pw], lhsT=v8_sb[g][:, 2 * p:2 * p + 2, :],
                                 rhs=P8[:, :, :pw], start=first, stop=last,
                                 perf_mode=DR, skip_group_check=True)
                nc.tensor.matmul(den_ps[:, :pw], lhsT=ones8,
                                 rhs=P8[:, :, :pw], start=first, stop=last,
                                 perf_mode=DR, skip_group_check=True)

            if TUNE["order_old_first"]:
                steps = ([(sc_old, fin_old, p) for p in (1, 0, 2, 3, 4, 5, 6, 7)] +
                         [(sc_rec, fin_rec, pr) for pr in range(2)])
            else:
                steps = ([(sc_rec, fin_rec, pr) for pr in range(2)] +
                         [(sc_old, fin_old, p) for p in range(8)])
            if TUNE["one_ahead"]:
                prev = None
                for idx, (scf, finf, arg) in enumerate(steps):
                    scp = scf(arg)
                    if idx % 2 == 1:
                        emit_job()
                    if prev is not None:
                        prev[0](prev[1], prev[2])
                    if idx == 6 and h + 1 < H:
                        emit_q(h + 1)
                    prev = (finf, arg, scp)
                prev[0](prev[1], prev[2])
            else:
                for idx, (scf, finf, arg) in enumerate(steps):
                    scp = scf(arg)
                    finf(arg, scp)
                    if idx == 6 and h + 1 < H:
                        emit_q(h + 1)

            # --- normalize
            drow = work.tile([1, R], f32, tag="drow")
            nc.vector.tensor_sub(drow, den_ps[0:1, :], npad_sb)
            rrow = work.tile([1, R], f32, tag="rrow")
            nc.vector.reciprocal_approx_fast(out=rrow, in_=drow)
            rbc = work.tile([128, R], f32, tag="rbc")
            nc.gpsimd.partition_broadcast(rbc, rrow)
            nc.vector.tensor_mul(ctxn[h], ctx_ps, rbc)

        emit_kv_rec_K(0)
        emit_kv_rec_V(0)
        emit_q(0)
        for b in range(NOB):
            emit_kv_old_K(0, b)
            emit_kv_old_V(0, b)
        if TUNE["kv_jobs"]:
            jobs.extend([
                lambda: emit_kv_old_K(1, 0), lambda: emit_kv_old_V(1, 0),
                lambda: emit_kv_old_K(1, 1), lambda: emit_kv_old_V(1, 1),
                lambda: emit_kv_old_K(1, 2), lambda: emit_kv_old_V(1, 2),
                lambda: emit_kv_old_K(1, 3), lambda: emit_kv_old_V(1, 3),
                lambda: emit_kv_rec_K(1), lambda: emit_kv_rec_V(1),
            ])
        else:
            emit_kv_rec_K(1)
            emit_kv_rec_V(1)
            for b in range(NOB):
                emit_kv_old_K(1, b)
                emit_kv_old_V(1, b)
        oacc = [kvp.tile([128, R], f32, name=f"oacc{ot}") for ot in range(HC)]

        def emit_oproj_half1(ots):
            for ot in ots:
                ops = psC.tile([128, R], f32, tag="ctx", name=f"oh{ot}")
                for hh in range(4):
                    nc.tensor.matmul(ops, lhsT=wo16_sb[:, hh, ts(ot, 128)],
                                     rhs=ctxn[hh], start=(hh == 0), stop=(hh == 3))
                nc.vector.tensor_copy(out=oacc[ot], in_=ops)

        in_attn[0] = True
        for h in range(H):
            emit_attn(h)
            if TUNE["o_split"] and h >= 4:
                emit_oproj_half1(range(2 * (h - 4), 2 * (h - 4) + 2))
        while jobs:
            emit_job()

        # ---------------- o_proj tail
        for ot in range(HC):
            ops = psC.tile([128, R], f32, tag="ctx", name=f"ops{ot}")
            hs = range(4, H) if TUNE["o_split"] else range(H)
            first = hs[0] if hasattr(hs, '__getitem__') else 4
            hs = list(hs)
            for h in hs:
                nc.tensor.matmul(ops, lhsT=wo16_sb[:, h, ts(ot, 128)],
                                 rhs=ctxn[h], start=(h == hs[0]), stop=(h == hs[-1]))
            ob = work.tile([128, R], f32, tag="ob")
            if TUNE["o_split"]:
                nc.vector.tensor_add(ob, ops, oacc[ot])
            else:
                nc.vector.tensor_copy(out=ob, in_=ops)
            nc.sync.dma_start(out=outT[ot], in_=ob)

    nc.compile()
    return nc


# ---------------------------------------------------------------- host side
def host_prep(cfg, x, wq, wk, wv, wo, pos):
    """x: [S, HID] f32, weights as in reference, pos: [S] int. Returns list of
    per-core input dicts."""
    R, W, HID, H, KVH, D, TH = (cfg["R"], cfg["W"], cfg["HID"], cfg["H"],
                                cfg["KVH"], cfg["D"], cfg["THETA"])
    KVW, HC, NKC = _derived(cfg)
    S = x.shape[0]
    ncores = S // R
    f8 = ml_dtypes.float8_e4m3
    inv_freq = (1.0 / TH ** (np.arange(0, D, 2, dtype=np.float64) / D))

    def pack_pm(wt, ncol, dt):
        a = wt.reshape(-1, 128, ncol)            # [chunks, 128, ncol]
        return np.ascontiguousarray(
            a.transpose(1, 0, 2).reshape(128, -1)).astype(dt)

    wq16 = pack_pm(wq.T, H * D, np.float16)
    wk16 = pack_pm(wk.T, KVH * D, np.float16)
    wv16 = pack_pm(wv.T, KVH * D, np.float16)
    wk8 = pack_pm(wk.T, KVH * D, f8)
    wv8 = pack_pm(wv.T, KVH * D, f8)
    wo16 = pack_pm(wo.T, HID, np.float16)

    in_maps = []
    for c in range(ncores):
        lo, hi = c * R - W, c * R + R
        pad = max(0, -lo)
        xw = np.zeros((KVW, HID), np.float32)
        xw[pad:] = x[max(lo, 0):hi]
        xTa = xw.T.reshape(HC, 128, KVW)                  # [c, p, j]
        parts = []
        for b0 in range(0, W, 512):
            blk = xTa[:, :, b0:b0 + 512].transpose(1, 0, 2)   # [p, c, j]
            parts.append(np.ascontiguousarray(blk).astype(f8).reshape(-1))
        xT8 = np.concatenate(parts)
        xT16 = np.ascontiguousarray(
            xTa[:, :, W:KVW].transpose(1, 0, 2).reshape(128, -1)).astype(np.float16)

        pw = np.zeros(KVW, np.float64)
        pw[pad:] = pos[max(lo, 0):hi].astype(np.float64)
        ang = pw[:, None] * inv_freq[None, :]          # [KVW, 64]
        ck, sk = np.cos(ang).T, np.sin(ang).T          # [64, KVW]
        cosK32 = np.concatenate([ck, ck], 0).astype(np.float32)
        sinK32 = np.concatenate([-sk, sk], 0).astype(np.float32)
        scale = 1.0 / np.sqrt(D)
        cosQ = (cosK32[:, W:] * scale).astype(np.float16)
        sinQ = (sinK32[:, W:] * scale).astype(np.float16)
        cosK = cosK32.astype(np.float16)
        sinK = sinK32.astype(np.float16)
        i_idx = np.arange(R, dtype=np.float32)
        npad = np.maximum(0.0, pad - 1.0 - i_idx)[None, :].astype(np.float32)

        in_maps.append(dict(xT8=xT8, xT16=xT16, wq16=wq16, wk16=wk16,
                            wv16=wv16, wk8=wk8, wv8=wv8, wo16=wo16,
                            cosK=cosK, sinK=sinK, cosQ=cosQ, sinQ=sinQ,
                            npadQ=npad))
    return in_maps


def assemble(cfg, outs):
    """outs: list of per-core outT arrays [HC, 128, R] -> [S, HID] f32."""
    R, HID = cfg["R"], cfg["HID"]
    blocks = [o.transpose(2, 0, 1).reshape(R, HID) for o in outs]
    return np.concatenate(blocks, 0).astype(np.float32)


_PROGRAM_CACHE = {}


def kernel(hidden_states, wq, wk, wv, wo, position_ids):
    from concourse.bass_utils import run_bass_kernel_spmd

    cfg = FULL_CFG
    x = np.asarray(hidden_states, np.float32)
    assert x.ndim == 3 and x.shape[0] == 1
    x2 = x[0]
    pos = np.asarray(position_ids)[0]
    in_maps = host_prep(cfg, x2, np.asarray(wq, np.float32),
                        np.asarray(wk, np.float32), np.asarray(wv, np.float32),
                        np.asarray(wo, np.float32), pos)
    key = "full"
    if key not in _PROGRAM_CACHE:
        _PROGRAM_CACHE[key] = build_program(cfg)
    nc = _PROGRAM_CACHE[key]
    res = run_bass_kernel_spmd(nc, in_maps, list(range(cfg["NCORES"])))
    outs = [res.results[i]["outT"] for i in range(cfg["NCORES"])]
    out = assemble(cfg, outs)
    return out.reshape(1, *out.shape)
